# revision 6
# baseline (speedup 1.0000x reference)
"""BWGNN (Beta-Wavelet GNN) forward on 8 Trainium2 NeuronCores.

Nodes are partitioned across 8 cores (12500 each); dense linears are
data-parallel. Each polynomial hop: scale rows by d^-1/2, AllGather the scaled
table, bulk random gather of in-edge src rows with dma_gather (int16 indices
against 4 src-range chunks, one SWDGE queue per chunk), strided vector reduce
per 128-node block (per-chunk degree-sorted lanes keep padding low), static
realign gather to join the 4 chunk partials, fused epilogue f' = f - agg*d^-1/2.

kernel(**inputs) takes FULL inputs and returns the FULL [N, 2] output.
"""
import os
import numpy as np

LAST_EXEC_NS = None

N = 100000
E = 1600000
IN = 128
H = 64
C = 2
THETAS = [[3.0, -3.0, 0.75], [0.0, 3.0, -1.5], [0.0, 0.0, 0.75]]

M = 8            # cores
NL = N // M      # 12500 nodes per core
NP = 128
NBLK = (NL + NP - 1) // NP   # 98
NPAD = NBLK * NP             # 12544
TRr = NPAD + 1               # per-rank table rows (zero row at NPAD)
NCHUNK = 4
CHROWS = 2 * TRr             # table rows per chunk (2 ranks) = 25002 < 32768
PADIDX = NPAD                # chunk-relative row of the first rank's zero row
MAX_IDX_CALL = 4096
MAX_NB = 16                  # max blocks per gather call (bounds reduce tile)


def _wrap_idx(flat):
    """int16 flat gather list -> [128, len/16] SBUF wrap (16 partitions, x8)."""
    iw = len(flat) // 16
    w = flat.reshape(iw, 16).T
    return np.ascontiguousarray(np.tile(w, (8, 1)).astype(np.int16))


def _host_prep(in_feat, src, dst):
    deg = np.bincount(dst, minlength=N)
    dinv = (1.0 / np.sqrt(np.maximum(deg, 1))).astype(np.float32)

    core_of = dst // NL
    chunk_of = src // (2 * NL)
    idx16 = ((src // NL - 2 * chunk_of) * TRr + src % NL).astype(np.int32)

    key = core_of * NCHUNK + chunk_of
    order = np.argsort(key, kind="stable")
    bounds = np.searchsorted(key[order], np.arange(M * NCHUNK + 1))

    K = np.zeros((NCHUNK, NBLK), dtype=np.int64)
    groups = {}
    degc_all = np.zeros((M, NCHUNK, NPAD), dtype=np.int64)
    ords = np.empty((M, NCHUNK, NPAD), dtype=np.int64)
    lanes = np.empty((M, NCHUNK, NPAD), dtype=np.int32)
    for c in range(M):
        for c4 in range(NCHUNK):
            g = order[bounds[c * NCHUNK + c4] : bounds[c * NCHUNK + c4 + 1]]
            groups[(c, c4)] = g
            dl = dst[g] - c * NL
            dc = np.bincount(dl, minlength=NPAD)
            degc_all[c, c4] = dc
            o = np.argsort(-dc, kind="stable")
            ords[c, c4] = o
            inv = np.empty(NPAD, dtype=np.int32)
            inv[o] = np.arange(NPAD, dtype=np.int32)
            lanes[c, c4] = inv
            K[c4] = np.maximum(K[c4], dc[o].reshape(NBLK, NP)[:, 0])

    # call schedule: per chunk, runs of equal-K consecutive blocks, capped
    calls = []  # (c4, kb, b0, nb, nidx)
    for c4 in range(NCHUNK):
        b = 0
        while b < NBLK:
            kb = int(K[c4][b])
            if kb == 0:
                b += 1
                continue
            e_ = b
            while e_ + 1 < NBLK and int(K[c4][e_ + 1]) == kb:
                e_ += 1
            maxnb = min(MAX_NB, max(1, MAX_IDX_CALL // (NP * kb)))
            while b <= e_:
                nb = min(maxnb, e_ - b + 1)
                calls.append((c4, kb, b, nb, NP * kb * nb))
                b += nb

    chunk_cols = [0] * NCHUNK
    call_col = []
    for (c4, kb, b0, nb, nidx) in calls:
        call_col.append(chunk_cols[c4])
        chunk_cols[c4] += nidx // 16

    idx_inputs = []
    ridx_inputs = []
    for c in range(M):
        per_chunk = []
        for c4 in range(NCHUNK):
            g = groups[(c, c4)]
            dl = dst[g] - c * NL
            lane = lanes[c, c4][dl].astype(np.int64)
            eorder = np.argsort(lane, kind="stable")
            ge = g[eorder]
            lane_s = lane[eorder]
            counts = degc_all[c, c4][ords[c, c4]]
            starts = np.zeros(NPAD + 1, dtype=np.int64)
            np.cumsum(counts, out=starts[1:])
            slot = np.arange(len(ge)) - starts[lane_s]
            flat = np.full(chunk_cols[c4] * 16, PADIDX, dtype=np.int32)
            blk = lane_s // NP
            j = lane_s % NP
            for ci, (cc4, kb, b0, nb, nidx) in enumerate(calls):
                if cc4 != c4:
                    continue
                sel = (blk >= b0) & (blk < b0 + nb) & (slot < kb)
                base = call_col[ci] * 16
                pos = base + ((blk[sel] - b0) * kb + slot[sel]) * NP + j[sel]
                flat[pos] = idx16[ge[sel]]
            per_chunk.append(_wrap_idx(flat.astype(np.int16)))
        idx_inputs.append(per_chunk)
        rflat = np.concatenate(
            [lanes[c, c4][:NPAD].astype(np.int16) for c4 in range(NCHUNK)]
        )
        ridx_inputs.append(_wrap_idx(rflat))

    xt_in, dinv_in = [], []
    for c in range(M):
        xt = np.zeros((IN, NPAD), dtype=np.float32)
        xt[:, :NL] = in_feat[c * NL : (c + 1) * NL].T
        xt_in.append(np.ascontiguousarray(xt))
        dv = np.ones(NPAD, dtype=np.float32)
        dv[:NL] = dinv[c * NL : (c + 1) * NL]
        dinv_in.append(np.ascontiguousarray(dv.reshape(NBLK, NP).T))
    return calls, call_col, chunk_cols, K, idx_inputs, ridx_inputs, xt_in, dinv_in


def _weights(W1, b1, W2, b2, W3, b3, W4, b4):
    Mk = [
        sum(THETAS[t][k] * W3[:, t * H : (t + 1) * H] for t in range(len(THETAS)))
        for k in range(3)
    ]
    return {
        "W1t": np.ascontiguousarray(W1.T.astype(np.float32)),
        "W2t": np.ascontiguousarray(W2.T.astype(np.float32)),
        "M0t": np.ascontiguousarray(Mk[0].T.astype(np.float32)),
        "M1t": np.ascontiguousarray(Mk[1].T.astype(np.float32)),
        "M2t": np.ascontiguousarray(Mk[2].T.astype(np.float32)),
        "W4t": np.ascontiguousarray(W4.T.astype(np.float32)),
        "b1r": b1.reshape(1, H).astype(np.float32),
        "b2r": b2.reshape(1, H).astype(np.float32),
        "b3r": b3.reshape(1, H).astype(np.float32),
        "b4r": b4.reshape(1, C).astype(np.float32),
    }


def _build_program(calls, call_col, chunk_cols, K):
    import concourse.bacc as bacc
    import concourse.mybir as mybir
    import concourse.tile as tile
    from concourse.library_config import mlp
    from concourse.masks import make_identity

    f32 = mybir.dt.float32
    i16 = mybir.dt.int16
    AX = mybir.AxisListType
    OP = mybir.AluOpType
    AF = mybir.ActivationFunctionType

    nc = bacc.Bacc(
        "TRN2", target_bir_lowering=False, debug=False, num_devices=M,
        num_swdge_queues=4,
    )

    xt_ext = nc.declare_dram_parameter("xt", [IN, NPAD], f32, isOutput=False)
    dinv_ext = nc.declare_dram_parameter("dinvT", [NP, NBLK], f32, isOutput=False)
    idx_ext = [
        nc.declare_dram_parameter(f"idx{c4}", [128, chunk_cols[c4]], i16, isOutput=False)
        for c4 in range(NCHUNK)
    ]
    ridx_ext = nc.declare_dram_parameter(
        "ridx", [128, NCHUNK * (NPAD // 16)], i16, isOutput=False
    )
    wshapes = [
        ("W1t", [IN, H]), ("W2t", [H, H]), ("M0t", [H, H]), ("M1t", [H, H]),
        ("M2t", [H, H]), ("W4t", [H, C]), ("b1r", [1, H]), ("b2r", [1, H]),
        ("b3r", [1, H]), ("b4r", [1, C]),
    ]
    wext = {nm: nc.declare_dram_parameter(nm, s, f32, isOutput=False) for nm, s in wshapes}
    out_ext = nc.declare_dram_parameter("out", [NP, NBLK * C], f32, isOutput=True)

    gloc = [nc.dram_tensor(f"g{h}loc", [TRr, H], f32) for h in range(2)]
    gfull = [
        nc.dram_tensor(f"g{h}full", [M * TRr, H], f32, addr_space="Shared")
        for h in range(2)
    ]
    aggdram = [nc.dram_tensor(f"agg{c4}", [NPAD, H], f32) for c4 in range(NCHUNK)]
    rg = [list(range(M))]

    with tile.TileContext(nc) as tc:
        with (
            tc.tile_pool(name="const", bufs=1) as cpool,
            tc.tile_pool(name="big", bufs=1) as bigpool,
            tc.tile_pool(name="xt", bufs=3) as xtpool,
            tc.tile_pool(name="work", bufs=3) as wpool,
            tc.tile_pool(name="idxp", bufs=2) as ipool,
            tc.tile_pool(name="gath", bufs=3) as gpool,
            tc.tile_pool(name="ridxp", bufs=1) as rpool,
            tc.tile_pool(name="ps", bufs=2, space="PSUM") as pspool,
        ):
            nc.gpsimd.load_library(mlp)

            W = {}
            for nm, s in wshapes:
                W[nm] = cpool.tile(list(s), f32, tag=nm, name=nm)
                nc.sync.dma_start(out=W[nm][:], in_=wext[nm][:])
            dinvT = cpool.tile([NP, NBLK], f32, tag="dinvT")
            nc.sync.dma_start(out=dinvT[:], in_=dinv_ext[:])
            ones_r = cpool.tile([1, NP], f32, tag="ones")
            nc.vector.memset(ones_r[:], 1.0)
            zrow = cpool.tile([1, H], f32, tag="zrow")
            nc.vector.memset(zrow[:], 0.0)
            zblk = cpool.tile([NP, H], f32, tag="zblk")
            nc.vector.memset(zblk[:], 0.0)
            ident = cpool.tile([NP, NP], f32, tag="ident")
            make_identity(nc, ident[:])
            ridx_t = cpool.tile([128, NCHUNK * (NPAD // 16)], i16, tag="ridx")
            nc.sync.dma_start(out=ridx_t[:], in_=ridx_ext[:])

            f0 = bigpool.tile([NP, NBLK * H], f32, tag="f0")
            f1 = bigpool.tile([NP, NBLK * H], f32, tag="f1")
            fX = bigpool.tile([NP, NBLK * H], f32, tag="fX")
            outb = bigpool.tile([NP, NBLK * C], f32, tag="outb")

            gloc_v = [g.ap()[0:NPAD, :].rearrange("(b j) d -> j b d", j=NP) for g in gloc]
            agg_v = [a.ap().rearrange("(b j) d -> j b d", j=NP) for a in aggdram]
            dbc = (
                dinvT[:]
                .rearrange("p (b o) -> p b o", o=1)
                .to_broadcast([NP, NBLK, H])
            )

            # ---------- phase A: L1 + L2, g0 table ----------
            for b in range(NBLK):
                xt = xtpool.tile([IN, NP], f32, tag="xt")
                nc.sync.dma_start(out=xt[:], in_=xt_ext[:, b * NP : (b + 1) * NP])
                ps1 = pspool.tile([NP, NP], f32, tag="pA", space="PSUM")
                nc.tensor.matmul(out=ps1[:H, :], lhsT=W["W1t"][:], rhs=xt[:], start=True, stop=False)
                nc.tensor.matmul(out=ps1[:H, :], lhsT=W["b1r"][:], rhs=ones_r[:], start=False, stop=True)
                h1t = wpool.tile([H, NP], f32, tag="h1t")
                nc.scalar.activation(h1t[:], ps1[:H, :], AF.Relu)
                ps2 = pspool.tile([NP, H], f32, tag="pB", space="PSUM")
                nc.tensor.matmul(out=ps2[:], lhsT=h1t[:], rhs=W["W2t"][:], start=True, stop=False)
                nc.tensor.matmul(out=ps2[:], lhsT=ones_r[:], rhs=W["b2r"][:], start=False, stop=True)
                f0b = f0[:, b * H : (b + 1) * H]
                nc.scalar.activation(f0b, ps2[:], AF.Relu)
                gb = wpool.tile([NP, H], f32, tag="gb")
                nc.vector.tensor_scalar(
                    out=gb[:], in0=f0b, scalar1=dinvT[:, b : b + 1], scalar2=None,
                    op0=OP.mult,
                )
                nc.sync.dma_start(out=gloc_v[0][:, b, :], in_=gb[:])
            nc.sync.dma_start(out=gloc[0][NPAD : NPAD + 1, :], in_=zrow[:])

            nc.gpsimd.collective_compute(
                "AllGather", OP.bypass, replica_groups=rg,
                ins=[gloc[0].ap().opt()], outs=[gfull[0].ap().opt()],
            )

            # ---------- two hops ----------
            for h in range(2):
                fprev = f0 if h == 0 else f1
                fout = f1 if h == 0 else fX
                table = gfull[h]
                cur_idx = {}
                for ci, (c4, kb, b0, nb, nidx) in enumerate(calls):
                    if c4 not in cur_idx:
                        it = ipool.tile([128, chunk_cols[c4]], i16, tag="idxc")
                        nc.sync.dma_start(out=it[:], in_=idx_ext[c4][:])
                        cur_idx[c4] = it
                    S = nidx // NP
                    dst_t = gpool.tile([NP, MAX_IDX_CALL // NP, H], f32, tag="gdst")
                    nc.gpsimd.dma_gather(
                        dst_t[:, :S, :],
                        table[c4 * CHROWS : (c4 + 1) * CHROWS, :],
                        cur_idx[c4][:, call_col[ci] : call_col[ci] + nidx // 16],
                        nidx,
                        nidx,
                        H,
                        single_packet=False,
                        queue_num=c4,
                    )
                    red = wpool.tile([NP, MAX_NB * H], f32, tag="red")
                    nc.vector.tensor_reduce(
                        out=red[:, : nb * H].rearrange("p (b d) -> p b d", b=nb),
                        in_=dst_t[:, :S, :].rearrange("p (b k) d -> p b d k", b=nb, k=kb),
                        axis=AX.X,
                        op=OP.add,
                    )
                    nc.sync.dma_start(
                        out=agg_v[c4][:, b0 : b0 + nb, :],
                        in_=red[:, : nb * H].rearrange("p (b d) -> p b d", b=nb),
                    )
                for c4 in range(NCHUNK):
                    for b in range(NBLK):
                        if K[c4][b] == 0:
                            nc.sync.dma_start(
                                out=aggdram[c4][b * NP : (b + 1) * NP, :], in_=zblk[:]
                            )
                for c4 in range(NCHUNK):
                    rt = rpool.tile([NP, NBLK, H], f32, tag="rt")
                    nc.gpsimd.dma_gather(
                        rt[:, :, :],
                        aggdram[c4][:, :],
                        ridx_t[:, c4 * (NPAD // 16) : (c4 + 1) * (NPAD // 16)],
                        NPAD,
                        NPAD,
                        H,
                        single_packet=False,
                        queue_num=c4,
                    )
                    rtf = rt[:].rearrange("p b d -> p (b d)")
                    if c4 == 0:
                        nc.vector.tensor_copy(fX[:], rtf)
                    else:
                        nc.vector.tensor_tensor(out=fX[:], in0=fX[:], in1=rtf, op=OP.add)
                fX3 = fX[:].rearrange("p (b d) -> p b d", b=NBLK)
                nc.vector.tensor_tensor(out=fX3, in0=fX3, in1=dbc, op=OP.mult)
                nc.vector.tensor_tensor(out=fout[:], in0=fprev[:], in1=fX[:], op=OP.subtract)
                if h == 0:
                    for b in range(NBLK):
                        gb = wpool.tile([NP, H], f32, tag="gb")
                        nc.vector.tensor_scalar(
                            out=gb[:], in0=f1[:, b * H : (b + 1) * H],
                            scalar1=dinvT[:, b : b + 1], scalar2=None, op0=OP.mult,
                        )
                        nc.sync.dma_start(out=gloc_v[1][:, b, :], in_=gb[:])
                    nc.sync.dma_start(out=gloc[1][NPAD : NPAD + 1, :], in_=zrow[:])
                    nc.gpsimd.collective_compute(
                        "AllGather", OP.bypass, replica_groups=rg,
                        ins=[gloc[1].ap().opt()], outs=[gfull[1].ap().opt()],
                    )

            # ---------- phase E: L3 + L4 ----------
            f2 = fX
            for bp in range(NBLK):
                ps3 = pspool.tile([NP, H], f32, tag="pB", space="PSUM")
                for k, (fk, mk) in enumerate(zip((f0, f1, f2), ("M0t", "M1t", "M2t"))):
                    pst = pspool.tile([NP, NP], f32, tag="pA", space="PSUM")
                    nc.tensor.transpose(
                        out=pst[:H, :], in_=fk[:, bp * H : (bp + 1) * H], identity=ident[:]
                    )
                    ft = wpool.tile([H, NP], f32, tag="ft")
                    nc.vector.tensor_copy(ft[:], pst[:H, :])
                    nc.tensor.matmul(
                        out=ps3[:], lhsT=ft[:], rhs=W[mk][:], start=(k == 0), stop=False,
                    )
                nc.tensor.matmul(
                    out=ps3[:], lhsT=ones_r[:], rhs=W["b3r"][:], start=False, stop=True
                )
                h3 = wpool.tile([NP, H], f32, tag="h3")
                nc.scalar.activation(h3[:], ps3[:], AF.Relu)
                psh = pspool.tile([NP, NP], f32, tag="pA", space="PSUM")
                nc.tensor.transpose(out=psh[:H, :], in_=h3[:], identity=ident[:])
                h3t = wpool.tile([H, NP], f32, tag="h3t")
                nc.vector.tensor_copy(h3t[:], psh[:H, :])
                ps4 = pspool.tile([NP, C], f32, tag="pC", space="PSUM")
                nc.tensor.matmul(out=ps4[:], lhsT=h3t[:], rhs=W["W4t"][:], start=True, stop=False)
                nc.tensor.matmul(out=ps4[:], lhsT=ones_r[:], rhs=W["b4r"][:], start=False, stop=True)
                nc.vector.tensor_copy(outb[:, bp * C : (bp + 1) * C], ps4[:])

            nc.sync.dma_start(out=out_ext[:], in_=outb[:])

    nc.compile()
    return nc


def kernel(**inputs):
    import concourse.bass_utils as bass_utils

    in_feat = np.asarray(inputs["in_feat"], dtype=np.float32)
    src = np.asarray(inputs["src"]).astype(np.int64)
    dst = np.asarray(inputs["dst"]).astype(np.int64)

    (calls, call_col, chunk_cols, K, idx_inputs, ridx_inputs, xt_in, dinv_in) = (
        _host_prep(in_feat, src, dst)
    )
    weights = _weights(
        np.asarray(inputs["W1"]), np.asarray(inputs["b1"]),
        np.asarray(inputs["W2"]), np.asarray(inputs["b2"]),
        np.asarray(inputs["W3"]), np.asarray(inputs["b3"]),
        np.asarray(inputs["W4"]), np.asarray(inputs["b4"]),
    )

    nc = _build_program(calls, call_col, chunk_cols, K)

    in_maps = []
    for c in range(M):
        im = {"xt": xt_in[c], "dinvT": dinv_in[c], "ridx": ridx_inputs[c]}
        for c4 in range(NCHUNK):
            im[f"idx{c4}"] = idx_inputs[c][c4]
        im.update(weights)
        in_maps.append(im)

    trace = bool(int(os.environ.get("BWGNN_TRACE", "0")))
    res = bass_utils.run_bass_kernel_spmd(nc, in_maps, list(range(M)), trace=trace)
    global LAST_EXEC_NS
    LAST_EXEC_NS = res.exec_time_ns

    full = np.empty((N, C), dtype=np.float32)
    for c in range(M):
        r = (
            res.results[c]["out"]
            .reshape(NP, NBLK, C)
            .transpose(1, 0, 2)
            .reshape(NPAD, C)
        )
        full[c * NL : (c + 1) * NL] = r[:NL]
    return full


# revision 7
# speedup vs baseline: 1.1217x; 1.1217x over previous
"""BWGNN (Beta-Wavelet GNN) forward on 8 Trainium2 NeuronCores.

Nodes are partitioned across 8 cores (12500 each); dense linears are
data-parallel. Each polynomial hop: scale rows by d^-1/2, AllGather the scaled
table, bulk random gather of in-edge src rows with dma_gather (int16 indices
against 4 src-range chunks, one SWDGE queue per chunk), strided vector reduce
per 128-node block (per-chunk degree-sorted lanes keep padding low), static
realign gather to join the 4 chunk partials, fused epilogue f' = f - agg*d^-1/2.

kernel(**inputs) takes FULL inputs and returns the FULL [N, 2] output.
"""
import os
import numpy as np

LAST_EXEC_NS = None

N = 100000
E = 1600000
IN = 128
H = 64
C = 2
THETAS = [[3.0, -3.0, 0.75], [0.0, 3.0, -1.5], [0.0, 0.0, 0.75]]

M = 8            # cores
NL = N // M      # 12500 nodes per core
NP = 128
NBLK = (NL + NP - 1) // NP   # 98
NPAD = NBLK * NP             # 12544
TRr = NPAD + 1               # per-rank table rows (zero row at NPAD)
NCHUNK = 4
CHROWS = 2 * TRr             # table rows per chunk (2 ranks) = 25002 < 32768
PADIDX = NPAD                # chunk-relative row of the first rank's zero row
MAX_IDX_CALL = 4096
MAX_NB = 16                  # max blocks per gather call (bounds reduce tile)


def _wrap_idx(flat):
    """int16 flat gather list -> [128, len/16] SBUF wrap (16 partitions, x8)."""
    iw = len(flat) // 16
    w = flat.reshape(iw, 16).T
    return np.ascontiguousarray(np.tile(w, (8, 1)).astype(np.int16))


def _host_prep(in_feat, src, dst):
    deg = np.bincount(dst, minlength=N)
    dinv = (1.0 / np.sqrt(np.maximum(deg, 1))).astype(np.float32)

    core_of = dst // NL
    chunk_of = src // (2 * NL)
    idx16 = ((src // NL - 2 * chunk_of) * TRr + src % NL).astype(np.int32)

    key = core_of * NCHUNK + chunk_of
    order = np.argsort(key, kind="stable")
    bounds = np.searchsorted(key[order], np.arange(M * NCHUNK + 1))

    K = np.zeros((NCHUNK, NBLK), dtype=np.int64)
    groups = {}
    degc_all = np.zeros((M, NCHUNK, NPAD), dtype=np.int64)
    ords = np.empty((M, NCHUNK, NPAD), dtype=np.int64)
    lanes = np.empty((M, NCHUNK, NPAD), dtype=np.int32)
    for c in range(M):
        for c4 in range(NCHUNK):
            g = order[bounds[c * NCHUNK + c4] : bounds[c * NCHUNK + c4 + 1]]
            groups[(c, c4)] = g
            dl = dst[g] - c * NL
            dc = np.bincount(dl, minlength=NPAD)
            degc_all[c, c4] = dc
            o = np.argsort(-dc, kind="stable")
            ords[c, c4] = o
            inv = np.empty(NPAD, dtype=np.int32)
            inv[o] = np.arange(NPAD, dtype=np.int32)
            lanes[c, c4] = inv
            K[c4] = np.maximum(K[c4], dc[o].reshape(NBLK, NP)[:, 0])

    # call schedule: per chunk, runs of equal-K consecutive blocks, capped
    calls = []  # (c4, kb, b0, nb, nidx)
    for c4 in range(NCHUNK):
        b = 0
        while b < NBLK:
            kb = int(K[c4][b])
            if kb == 0:
                b += 1
                continue
            e_ = b
            while e_ + 1 < NBLK and int(K[c4][e_ + 1]) == kb:
                e_ += 1
            maxnb = min(MAX_NB, max(1, MAX_IDX_CALL // (NP * kb)))
            while b <= e_:
                nb = min(maxnb, e_ - b + 1)
                calls.append((c4, kb, b, nb, NP * kb * nb))
                b += nb

    chunk_cols = [0] * NCHUNK
    call_col = []
    for (c4, kb, b0, nb, nidx) in calls:
        call_col.append(chunk_cols[c4])
        chunk_cols[c4] += nidx // 16

    idx_inputs = []
    ridx_inputs = []
    for c in range(M):
        per_chunk = []
        for c4 in range(NCHUNK):
            g = groups[(c, c4)]
            dl = dst[g] - c * NL
            lane = lanes[c, c4][dl].astype(np.int64)
            eorder = np.argsort(lane, kind="stable")
            ge = g[eorder]
            lane_s = lane[eorder]
            counts = degc_all[c, c4][ords[c, c4]]
            starts = np.zeros(NPAD + 1, dtype=np.int64)
            np.cumsum(counts, out=starts[1:])
            slot = np.arange(len(ge)) - starts[lane_s]
            flat = np.full(chunk_cols[c4] * 16, PADIDX, dtype=np.int32)
            blk = lane_s // NP
            j = lane_s % NP
            for ci, (cc4, kb, b0, nb, nidx) in enumerate(calls):
                if cc4 != c4:
                    continue
                sel = (blk >= b0) & (blk < b0 + nb) & (slot < kb)
                base = call_col[ci] * 16
                pos = base + ((blk[sel] - b0) * kb + slot[sel]) * NP + j[sel]
                flat[pos] = idx16[ge[sel]]
            per_chunk.append(_wrap_idx(flat.astype(np.int16)))
        idx_inputs.append(per_chunk)
        rflat = np.concatenate(
            [lanes[c, c4][:NPAD].astype(np.int16) for c4 in range(NCHUNK)]
        )
        ridx_inputs.append(_wrap_idx(rflat))

    xt_in, dinv_in = [], []
    for c in range(M):
        xt = np.zeros((IN, NPAD), dtype=np.float32)
        xt[:, :NL] = in_feat[c * NL : (c + 1) * NL].T
        xt_in.append(np.ascontiguousarray(xt))
        dv = np.ones(NPAD, dtype=np.float32)
        dv[:NL] = dinv[c * NL : (c + 1) * NL]
        dinv_in.append(np.ascontiguousarray(dv.reshape(NBLK, NP).T))
    return calls, call_col, chunk_cols, K, idx_inputs, ridx_inputs, xt_in, dinv_in


def _weights(W1, b1, W2, b2, W3, b3, W4, b4):
    Mk = [
        sum(THETAS[t][k] * W3[:, t * H : (t + 1) * H] for t in range(len(THETAS)))
        for k in range(3)
    ]
    return {
        "W1t": np.ascontiguousarray(W1.T.astype(np.float32)),
        "W2t": np.ascontiguousarray(W2.T.astype(np.float32)),
        "M0t": np.ascontiguousarray(Mk[0].T.astype(np.float32)),
        "M1t": np.ascontiguousarray(Mk[1].T.astype(np.float32)),
        "M2t": np.ascontiguousarray(Mk[2].T.astype(np.float32)),
        "W4t": np.ascontiguousarray(W4.T.astype(np.float32)),
        "b1r": b1.reshape(1, H).astype(np.float32),
        "b2r": b2.reshape(1, H).astype(np.float32),
        "b3r": b3.reshape(1, H).astype(np.float32),
        "b4r": b4.reshape(1, C).astype(np.float32),
    }


def _build_program(calls, call_col, chunk_cols, K):
    import concourse.bacc as bacc
    import concourse.mybir as mybir
    import concourse.tile as tile
    from concourse.library_config import mlp
    from concourse.masks import make_identity

    f32 = mybir.dt.float32
    i16 = mybir.dt.int16
    AX = mybir.AxisListType
    OP = mybir.AluOpType
    AF = mybir.ActivationFunctionType

    nc = bacc.Bacc(
        "TRN2", target_bir_lowering=False, debug=False, num_devices=M,
        num_swdge_queues=4,
    )

    xt_ext = nc.declare_dram_parameter("xt", [IN, NPAD], f32, isOutput=False)
    dinv_ext = nc.declare_dram_parameter("dinvT", [NP, NBLK], f32, isOutput=False)
    idx_ext = [
        nc.declare_dram_parameter(f"idx{c4}", [128, chunk_cols[c4]], i16, isOutput=False)
        for c4 in range(NCHUNK)
    ]
    ridx_ext = nc.declare_dram_parameter(
        "ridx", [128, NCHUNK * (NPAD // 16)], i16, isOutput=False
    )
    wshapes = [
        ("W1t", [IN, H]), ("W2t", [H, H]), ("M0t", [H, H]), ("M1t", [H, H]),
        ("M2t", [H, H]), ("W4t", [H, C]), ("b1r", [1, H]), ("b2r", [1, H]),
        ("b3r", [1, H]), ("b4r", [1, C]),
    ]
    wext = {nm: nc.declare_dram_parameter(nm, s, f32, isOutput=False) for nm, s in wshapes}
    out_ext = nc.declare_dram_parameter("out", [NP, NBLK * C], f32, isOutput=True)

    gloc = [nc.dram_tensor(f"g{h}loc", [TRr, H], f32) for h in range(2)]
    gfull = [
        nc.dram_tensor(f"g{h}full", [M * TRr, H], f32, addr_space="Shared")
        for h in range(2)
    ]
    aggdram = [nc.dram_tensor(f"agg{c4}", [NPAD, H], f32) for c4 in range(NCHUNK)]
    rg = [list(range(M))]

    with tile.TileContext(nc) as tc:
        with (
            tc.tile_pool(name="const", bufs=1) as cpool,
            tc.tile_pool(name="big", bufs=1) as bigpool,
            tc.tile_pool(name="xt", bufs=3) as xtpool,
            tc.tile_pool(name="work", bufs=3) as wpool,
            tc.tile_pool(name="idxp", bufs=2) as ipool,
            tc.tile_pool(name="gath", bufs=8) as gpool,
            tc.tile_pool(name="ridxp", bufs=1) as rpool,
            tc.tile_pool(name="ps", bufs=2, space="PSUM") as pspool,
        ):
            nc.gpsimd.load_library(mlp)

            W = {}
            for nm, s in wshapes:
                W[nm] = cpool.tile(list(s), f32, tag=nm, name=nm)
                nc.sync.dma_start(out=W[nm][:], in_=wext[nm][:])
            dinvT = cpool.tile([NP, NBLK], f32, tag="dinvT")
            nc.sync.dma_start(out=dinvT[:], in_=dinv_ext[:])
            ones_r = cpool.tile([1, NP], f32, tag="ones")
            nc.vector.memset(ones_r[:], 1.0)
            zrow = cpool.tile([1, H], f32, tag="zrow")
            nc.vector.memset(zrow[:], 0.0)
            zblk = cpool.tile([NP, H], f32, tag="zblk")
            nc.vector.memset(zblk[:], 0.0)
            ident = cpool.tile([NP, NP], f32, tag="ident")
            make_identity(nc, ident[:])
            ridx_t = cpool.tile([128, NCHUNK * (NPAD // 16)], i16, tag="ridx")
            nc.sync.dma_start(out=ridx_t[:], in_=ridx_ext[:])

            f0 = bigpool.tile([NP, NBLK * H], f32, tag="f0")
            f1 = bigpool.tile([NP, NBLK * H], f32, tag="f1")
            fX = bigpool.tile([NP, NBLK * H], f32, tag="fX")
            outb = bigpool.tile([NP, NBLK * C], f32, tag="outb")

            gloc_v = [g.ap()[0:NPAD, :].rearrange("(b j) d -> j b d", j=NP) for g in gloc]
            agg_v = [a.ap().rearrange("(b j) d -> j b d", j=NP) for a in aggdram]
            dbc = (
                dinvT[:]
                .rearrange("p (b o) -> p b o", o=1)
                .to_broadcast([NP, NBLK, H])
            )

            # ---------- phase A: L1 + L2, g0 table ----------
            for b in range(NBLK):
                xt = xtpool.tile([IN, NP], f32, tag="xt")
                nc.sync.dma_start(out=xt[:], in_=xt_ext[:, b * NP : (b + 1) * NP])
                ps1 = pspool.tile([NP, NP], f32, tag="pA", space="PSUM")
                nc.tensor.matmul(out=ps1[:H, :], lhsT=W["W1t"][:], rhs=xt[:], start=True, stop=False)
                nc.tensor.matmul(out=ps1[:H, :], lhsT=W["b1r"][:], rhs=ones_r[:], start=False, stop=True)
                h1t = wpool.tile([H, NP], f32, tag="h1t")
                nc.vector.tensor_scalar(out=h1t[:], in0=ps1[:H, :], scalar1=0.0, scalar2=None, op0=OP.max)
                ps2 = pspool.tile([NP, H], f32, tag="pB", space="PSUM")
                nc.tensor.matmul(out=ps2[:], lhsT=h1t[:], rhs=W["W2t"][:], start=True, stop=False)
                nc.tensor.matmul(out=ps2[:], lhsT=ones_r[:], rhs=W["b2r"][:], start=False, stop=True)
                f0b = f0[:, b * H : (b + 1) * H]
                nc.vector.tensor_scalar(out=f0b, in0=ps2[:], scalar1=0.0, scalar2=None, op0=OP.max)
                gb = wpool.tile([NP, H], f32, tag="gb")
                nc.vector.tensor_scalar(
                    out=gb[:], in0=f0b, scalar1=dinvT[:, b : b + 1], scalar2=None,
                    op0=OP.mult,
                )
                nc.scalar.dma_start(out=gloc_v[0][:, b, :], in_=gb[:])
            nc.sync.dma_start(out=gloc[0][NPAD : NPAD + 1, :], in_=zrow[:])

            nc.gpsimd.collective_compute(
                "AllGather", OP.bypass, replica_groups=rg,
                ins=[gloc[0].ap().opt()], outs=[gfull[0].ap().opt()],
            )

            # ---------- two hops ----------
            for h in range(2):
                fprev = f0 if h == 0 else f1
                fout = f1 if h == 0 else fX
                table = gfull[h]
                cur_idx = {}
                for ci, (c4, kb, b0, nb, nidx) in enumerate(calls):
                    if c4 not in cur_idx:
                        it = ipool.tile([128, chunk_cols[c4]], i16, tag="idxc")
                        nc.sync.dma_start(out=it[:], in_=idx_ext[c4][:])
                        cur_idx[c4] = it
                    S = nidx // NP
                    dst_t = gpool.tile([NP, MAX_IDX_CALL // NP, H], f32, tag="gdst")
                    nc.gpsimd.dma_gather(
                        dst_t[:, :S, :],
                        table[c4 * CHROWS : (c4 + 1) * CHROWS, :],
                        cur_idx[c4][:, call_col[ci] : call_col[ci] + nidx // 16],
                        nidx,
                        nidx,
                        H,
                        single_packet=False,
                        queue_num=c4,
                    )
                    red = wpool.tile([NP, MAX_NB * H], f32, tag="red")
                    nc.vector.tensor_reduce(
                        out=red[:, : nb * H].rearrange("p (b d) -> p b d", b=nb),
                        in_=dst_t[:, :S, :].rearrange("p (b k) d -> p b d k", b=nb, k=kb),
                        axis=AX.X,
                        op=OP.add,
                    )
                    nc.scalar.dma_start(
                        out=agg_v[c4][:, b0 : b0 + nb, :],
                        in_=red[:, : nb * H].rearrange("p (b d) -> p b d", b=nb),
                    )
                for c4 in range(NCHUNK):
                    for b in range(NBLK):
                        if K[c4][b] == 0:
                            nc.sync.dma_start(
                                out=aggdram[c4][b * NP : (b + 1) * NP, :], in_=zblk[:]
                            )
                for c4 in range(NCHUNK):
                    rt = rpool.tile([NP, NBLK, H], f32, tag="rt")
                    nc.gpsimd.dma_gather(
                        rt[:, :, :],
                        aggdram[c4][:, :],
                        ridx_t[:, c4 * (NPAD // 16) : (c4 + 1) * (NPAD // 16)],
                        NPAD,
                        NPAD,
                        H,
                        single_packet=False,
                        queue_num=c4,
                    )
                    rtf = rt[:].rearrange("p b d -> p (b d)")
                    if c4 == 0:
                        nc.vector.tensor_copy(fX[:], rtf)
                    else:
                        nc.vector.tensor_tensor(out=fX[:], in0=fX[:], in1=rtf, op=OP.add)
                fX3 = fX[:].rearrange("p (b d) -> p b d", b=NBLK)
                nc.vector.tensor_tensor(out=fX3, in0=fX3, in1=dbc, op=OP.mult)
                nc.vector.tensor_tensor(out=fout[:], in0=fprev[:], in1=fX[:], op=OP.subtract)
                if h == 0:
                    for b in range(NBLK):
                        gb = wpool.tile([NP, H], f32, tag="gb")
                        nc.vector.tensor_scalar(
                            out=gb[:], in0=f1[:, b * H : (b + 1) * H],
                            scalar1=dinvT[:, b : b + 1], scalar2=None, op0=OP.mult,
                        )
                        nc.scalar.dma_start(out=gloc_v[1][:, b, :], in_=gb[:])
                    nc.sync.dma_start(out=gloc[1][NPAD : NPAD + 1, :], in_=zrow[:])
                    nc.gpsimd.collective_compute(
                        "AllGather", OP.bypass, replica_groups=rg,
                        ins=[gloc[1].ap().opt()], outs=[gfull[1].ap().opt()],
                    )

            # ---------- phase E: L3 + L4 ----------
            f2 = fX
            for bp in range(NBLK):
                ps3 = pspool.tile([NP, H], f32, tag="pB", space="PSUM")
                for k, (fk, mk) in enumerate(zip((f0, f1, f2), ("M0t", "M1t", "M2t"))):
                    pst = pspool.tile([NP, NP], f32, tag="pA", space="PSUM")
                    nc.tensor.transpose(
                        out=pst[:H, :], in_=fk[:, bp * H : (bp + 1) * H], identity=ident[:]
                    )
                    ft = wpool.tile([H, NP], f32, tag="ft")
                    nc.vector.tensor_copy(ft[:], pst[:H, :])
                    nc.tensor.matmul(
                        out=ps3[:], lhsT=ft[:], rhs=W[mk][:], start=(k == 0), stop=False,
                    )
                nc.tensor.matmul(
                    out=ps3[:], lhsT=ones_r[:], rhs=W["b3r"][:], start=False, stop=True
                )
                h3 = wpool.tile([NP, H], f32, tag="h3")
                nc.vector.tensor_scalar(out=h3[:], in0=ps3[:], scalar1=0.0, scalar2=None, op0=OP.max)
                psh = pspool.tile([NP, NP], f32, tag="pA", space="PSUM")
                nc.tensor.transpose(out=psh[:H, :], in_=h3[:], identity=ident[:])
                h3t = wpool.tile([H, NP], f32, tag="h3t")
                nc.vector.tensor_copy(h3t[:], psh[:H, :])
                ps4 = pspool.tile([NP, C], f32, tag="pC", space="PSUM")
                nc.tensor.matmul(out=ps4[:], lhsT=h3t[:], rhs=W["W4t"][:], start=True, stop=False)
                nc.tensor.matmul(out=ps4[:], lhsT=ones_r[:], rhs=W["b4r"][:], start=False, stop=True)
                nc.vector.tensor_copy(outb[:, bp * C : (bp + 1) * C], ps4[:])

            nc.sync.dma_start(out=out_ext[:], in_=outb[:])

    nc.compile()
    return nc


def kernel(**inputs):
    import concourse.bass_utils as bass_utils

    in_feat = np.asarray(inputs["in_feat"], dtype=np.float32)
    src = np.asarray(inputs["src"]).astype(np.int64)
    dst = np.asarray(inputs["dst"]).astype(np.int64)

    (calls, call_col, chunk_cols, K, idx_inputs, ridx_inputs, xt_in, dinv_in) = (
        _host_prep(in_feat, src, dst)
    )
    weights = _weights(
        np.asarray(inputs["W1"]), np.asarray(inputs["b1"]),
        np.asarray(inputs["W2"]), np.asarray(inputs["b2"]),
        np.asarray(inputs["W3"]), np.asarray(inputs["b3"]),
        np.asarray(inputs["W4"]), np.asarray(inputs["b4"]),
    )

    nc = _build_program(calls, call_col, chunk_cols, K)

    in_maps = []
    for c in range(M):
        im = {"xt": xt_in[c], "dinvT": dinv_in[c], "ridx": ridx_inputs[c]}
        for c4 in range(NCHUNK):
            im[f"idx{c4}"] = idx_inputs[c][c4]
        im.update(weights)
        in_maps.append(im)

    trace = bool(int(os.environ.get("BWGNN_TRACE", "0")))
    res = bass_utils.run_bass_kernel_spmd(nc, in_maps, list(range(M)), trace=trace)
    global LAST_EXEC_NS
    LAST_EXEC_NS = res.exec_time_ns

    full = np.empty((N, C), dtype=np.float32)
    for c in range(M):
        r = (
            res.results[c]["out"]
            .reshape(NP, NBLK, C)
            .transpose(1, 0, 2)
            .reshape(NPAD, C)
        )
        full[c * NL : (c + 1) * NL] = r[:NL]
    return full


# revision 9
# speedup vs baseline: 1.7459x; 1.5565x over previous
"""BWGNN (Beta-Wavelet GNN) forward on 8 Trainium2 NeuronCores.

Nodes are partitioned across 8 cores (12500 each); dense linears are
data-parallel. Each polynomial hop: scale rows by d^-1/2, AllGather the scaled
table, bulk random gather of in-edge src rows with dma_gather (int16 indices
against 4 src-range chunks, one SWDGE queue per chunk), strided vector reduce
per 128-node block (per-chunk degree-sorted lanes keep padding low), static
realign gather to join the 4 chunk partials, fused epilogue f' = f - agg*d^-1/2.

kernel(**inputs) takes FULL inputs and returns the FULL [N, 2] output.
"""
import os
import numpy as np

LAST_EXEC_NS = None

N = 100000
E = 1600000
IN = 128
H = 64
C = 2
THETAS = [[3.0, -3.0, 0.75], [0.0, 3.0, -1.5], [0.0, 0.0, 0.75]]

M = 8            # cores
NL = N // M      # 12500 nodes per core
NP = 128
NBLK = (NL + NP - 1) // NP   # 98
NPAD = NBLK * NP             # 12544
TRr = NPAD + 1               # per-rank table rows (zero row at NPAD)
NCHUNK = 4
CHROWS = 2 * TRr             # table rows per chunk (2 ranks) = 25002 < 32768
PADIDX = NPAD                # chunk-relative row of the first rank's zero row
MAX_IDX_CALL = 4096
MAX_NB = 16                  # max blocks per gather call (bounds reduce tile)


def _wrap_idx(flat):
    """int16 flat gather list -> [128, len/16] SBUF wrap (16 partitions, x8)."""
    iw = len(flat) // 16
    w = flat.reshape(iw, 16).T
    return np.ascontiguousarray(np.tile(w, (8, 1)).astype(np.int16))


def _host_prep(in_feat, src, dst):
    deg = np.bincount(dst, minlength=N)
    dinv = (1.0 / np.sqrt(np.maximum(deg, 1))).astype(np.float32)

    core_of = dst // NL
    chunk_of = src // (2 * NL)
    idx16 = ((src // NL - 2 * chunk_of) * TRr + src % NL).astype(np.int32)

    key = core_of * NCHUNK + chunk_of
    order = np.argsort(key, kind="stable")
    bounds = np.searchsorted(key[order], np.arange(M * NCHUNK + 1))

    K = np.zeros((NCHUNK, NBLK), dtype=np.int64)
    groups = {}
    degc_all = np.zeros((M, NCHUNK, NPAD), dtype=np.int64)
    ords = np.empty((M, NCHUNK, NPAD), dtype=np.int64)
    lanes = np.empty((M, NCHUNK, NPAD), dtype=np.int32)
    for c in range(M):
        for c4 in range(NCHUNK):
            g = order[bounds[c * NCHUNK + c4] : bounds[c * NCHUNK + c4 + 1]]
            groups[(c, c4)] = g
            dl = dst[g] - c * NL
            dc = np.bincount(dl, minlength=NPAD)
            degc_all[c, c4] = dc
            o = np.argsort(-dc, kind="stable")
            ords[c, c4] = o
            inv = np.empty(NPAD, dtype=np.int32)
            inv[o] = np.arange(NPAD, dtype=np.int32)
            lanes[c, c4] = inv
            K[c4] = np.maximum(K[c4], dc[o].reshape(NBLK, NP)[:, 0])

    # call schedule: per chunk, runs of equal-K consecutive blocks, capped
    calls = []  # (c4, kb, b0, nb, nidx)
    for c4 in range(NCHUNK):
        b = 0
        while b < NBLK:
            kb = int(K[c4][b])
            if kb == 0:
                b += 1
                continue
            e_ = b
            while e_ + 1 < NBLK and int(K[c4][e_ + 1]) == kb:
                e_ += 1
            maxnb = min(MAX_NB, max(1, MAX_IDX_CALL // (NP * kb)))
            while b <= e_:
                nb = min(maxnb, e_ - b + 1)
                calls.append((c4, kb, b, nb, NP * kb * nb))
                b += nb

    chunk_cols = [0] * NCHUNK
    call_col = []
    for (c4, kb, b0, nb, nidx) in calls:
        call_col.append(chunk_cols[c4])
        chunk_cols[c4] += nidx // 16

    idx_inputs = []
    ridx_inputs = []
    for c in range(M):
        per_chunk = []
        for c4 in range(NCHUNK):
            g = groups[(c, c4)]
            dl = dst[g] - c * NL
            lane = lanes[c, c4][dl].astype(np.int64)
            eorder = np.argsort(lane, kind="stable")
            ge = g[eorder]
            lane_s = lane[eorder]
            counts = degc_all[c, c4][ords[c, c4]]
            starts = np.zeros(NPAD + 1, dtype=np.int64)
            np.cumsum(counts, out=starts[1:])
            slot = np.arange(len(ge)) - starts[lane_s]
            flat = np.full(chunk_cols[c4] * 16, PADIDX, dtype=np.int32)
            blk = lane_s // NP
            j = lane_s % NP
            for ci, (cc4, kb, b0, nb, nidx) in enumerate(calls):
                if cc4 != c4:
                    continue
                sel = (blk >= b0) & (blk < b0 + nb) & (slot < kb)
                base = call_col[ci] * 16
                pos = base + ((blk[sel] - b0) * kb + slot[sel]) * NP + j[sel]
                flat[pos] = idx16[ge[sel]]
            per_chunk.append(_wrap_idx(flat.astype(np.int16)))
        idx_inputs.append(per_chunk)
        rflat = np.concatenate(
            [lanes[c, c4][:NPAD].astype(np.int16) for c4 in range(NCHUNK)]
        )
        ridx_inputs.append(_wrap_idx(rflat))

    xt_in, dinv_in = [], []
    for c in range(M):
        xt = np.zeros((IN, NPAD), dtype=np.float32)
        xt[:, :NL] = in_feat[c * NL : (c + 1) * NL].T
        xt_in.append(np.ascontiguousarray(xt))
        dv = np.ones(NPAD, dtype=np.float32)
        dv[:NL] = dinv[c * NL : (c + 1) * NL]
        dinv_in.append(np.ascontiguousarray(dv.reshape(NBLK, NP).T))
    return calls, call_col, chunk_cols, K, idx_inputs, ridx_inputs, xt_in, dinv_in


def _weights(W1, b1, W2, b2, W3, b3, W4, b4):
    Mk = [
        sum(THETAS[t][k] * W3[:, t * H : (t + 1) * H] for t in range(len(THETAS)))
        for k in range(3)
    ]
    return {
        "W1t": np.ascontiguousarray(W1.T.astype(np.float32)),
        "W2t": np.ascontiguousarray(W2.T.astype(np.float32)),
        "M0t": np.ascontiguousarray(Mk[0].T.astype(np.float32)),
        "M1t": np.ascontiguousarray(Mk[1].T.astype(np.float32)),
        "M2t": np.ascontiguousarray(Mk[2].T.astype(np.float32)),
        "W4t": np.ascontiguousarray(W4.T.astype(np.float32)),
        "b1r": b1.reshape(1, H).astype(np.float32),
        "b2r": b2.reshape(1, H).astype(np.float32),
        "b3r": b3.reshape(1, H).astype(np.float32),
        "b4r": b4.reshape(1, C).astype(np.float32),
    }


def _build_program(calls, call_col, chunk_cols, K):
    import concourse.bacc as bacc
    import concourse.mybir as mybir
    import concourse.tile as tile
    from concourse.library_config import mlp
    from concourse.masks import make_identity

    f32 = mybir.dt.float32
    i16 = mybir.dt.int16
    AX = mybir.AxisListType
    OP = mybir.AluOpType
    AF = mybir.ActivationFunctionType

    nc = bacc.Bacc(
        "TRN2", target_bir_lowering=False, debug=False, num_devices=M,
        num_swdge_queues=4,
    )

    xt_ext = nc.declare_dram_parameter("xt", [IN, NPAD], f32, isOutput=False)
    dinv_ext = nc.declare_dram_parameter("dinvT", [NP, NBLK], f32, isOutput=False)
    idx_ext = [
        nc.declare_dram_parameter(f"idx{c4}", [128, chunk_cols[c4]], i16, isOutput=False)
        for c4 in range(NCHUNK)
    ]
    ridx_ext = nc.declare_dram_parameter(
        "ridx", [128, NCHUNK * (NPAD // 16)], i16, isOutput=False
    )
    wshapes = [
        ("W1t", [IN, H]), ("W2t", [H, H]), ("M0t", [H, H]), ("M1t", [H, H]),
        ("M2t", [H, H]), ("W4t", [H, C]), ("b1r", [1, H]), ("b2r", [1, H]),
        ("b3r", [1, H]), ("b4r", [1, C]),
    ]
    wext = {nm: nc.declare_dram_parameter(nm, s, f32, isOutput=False) for nm, s in wshapes}
    out_ext = nc.declare_dram_parameter("out", [NP, NBLK * C], f32, isOutput=True)

    gloc = [nc.dram_tensor(f"g{h}loc", [TRr, H], f32) for h in range(2)]
    gfull = [
        nc.dram_tensor(f"g{h}full", [M * TRr, H], f32, addr_space="Shared")
        for h in range(2)
    ]
    aggdram = [nc.dram_tensor(f"agg{c4}", [NPAD, H], f32) for c4 in range(NCHUNK)]
    rg = [list(range(M))]

    with tile.TileContext(nc) as tc:
        with (
            tc.tile_pool(name="const", bufs=1) as cpool,
            tc.tile_pool(name="big", bufs=1) as bigpool,
            tc.tile_pool(name="xt", bufs=3) as xtpool,
            tc.tile_pool(name="work", bufs=3) as wpool,
            tc.tile_pool(name="idxp", bufs=1) as ipool,
            tc.tile_pool(name="gath", bufs=6) as gpool,
            tc.tile_pool(name="ridxp", bufs=2) as rpool,
            tc.tile_pool(name="ps", bufs=2, space="PSUM") as pspool,
        ):
            nc.gpsimd.load_library(mlp)

            W = {}
            for nm, s in wshapes:
                W[nm] = cpool.tile(list(s), f32, tag=nm, name=nm)
                nc.sync.dma_start(out=W[nm][:], in_=wext[nm][:])
            dinvT = cpool.tile([NP, NBLK], f32, tag="dinvT")
            nc.sync.dma_start(out=dinvT[:], in_=dinv_ext[:])
            ones_r = cpool.tile([1, NP], f32, tag="ones")
            nc.vector.memset(ones_r[:], 1.0)
            zrow = cpool.tile([1, H], f32, tag="zrow")
            nc.vector.memset(zrow[:], 0.0)
            zblk = cpool.tile([NP, H], f32, tag="zblk")
            nc.vector.memset(zblk[:], 0.0)
            ident = cpool.tile([NP, NP], f32, tag="ident")
            make_identity(nc, ident[:])
            ridx_t = cpool.tile([128, NCHUNK * (NPAD // 16)], i16, tag="ridx")
            nc.sync.dma_start(out=ridx_t[:], in_=ridx_ext[:])

            f0 = bigpool.tile([NP, NBLK * H], f32, tag="f0")
            f1 = bigpool.tile([NP, NBLK * H], f32, tag="f1")
            fX = bigpool.tile([NP, NBLK * H], f32, tag="fX")
            outb = bigpool.tile([NP, NBLK * C], f32, tag="outb")

            gloc_v = [g.ap()[0:NPAD, :].rearrange("(b j) d -> j b d", j=NP) for g in gloc]
            agg_v = [a.ap().rearrange("(b j) d -> j b d", j=NP) for a in aggdram]
            dbc = (
                dinvT[:]
                .rearrange("p (b o) -> p b o", o=1)
                .to_broadcast([NP, NBLK, H])
            )

            # ---------- phase A: L1 + L2, g0 table ----------
            for b in range(NBLK):
                xt = xtpool.tile([IN, NP], f32, tag="xt")
                nc.sync.dma_start(out=xt[:], in_=xt_ext[:, b * NP : (b + 1) * NP])
                ps1 = pspool.tile([NP, NP], f32, tag="pA", space="PSUM")
                nc.tensor.matmul(out=ps1[:H, :], lhsT=W["W1t"][:], rhs=xt[:], start=True, stop=False)
                nc.tensor.matmul(out=ps1[:H, :], lhsT=W["b1r"][:], rhs=ones_r[:], start=False, stop=True)
                h1t = wpool.tile([H, NP], f32, tag="h1t")
                nc.vector.tensor_scalar(out=h1t[:], in0=ps1[:H, :], scalar1=0.0, scalar2=None, op0=OP.max)
                ps2 = pspool.tile([NP, H], f32, tag="pB", space="PSUM")
                nc.tensor.matmul(out=ps2[:], lhsT=h1t[:], rhs=W["W2t"][:], start=True, stop=False)
                nc.tensor.matmul(out=ps2[:], lhsT=ones_r[:], rhs=W["b2r"][:], start=False, stop=True)
                f0b = f0[:, b * H : (b + 1) * H]
                nc.vector.tensor_scalar(out=f0b, in0=ps2[:], scalar1=0.0, scalar2=None, op0=OP.max)
                gb = wpool.tile([NP, H], f32, tag="gb")
                nc.vector.tensor_scalar(
                    out=gb[:], in0=f0b, scalar1=dinvT[:, b : b + 1], scalar2=None,
                    op0=OP.mult,
                )
                nc.scalar.dma_start(out=gloc_v[0][:, b, :], in_=gb[:])
            nc.sync.dma_start(out=gloc[0][NPAD : NPAD + 1, :], in_=zrow[:])

            nc.gpsimd.collective_compute(
                "AllGather", OP.bypass, replica_groups=rg,
                ins=[gloc[0].ap().opt()], outs=[gfull[0].ap().opt()],
            )

            # ---------- two hops ----------
            for h in range(2):
                fprev = f0 if h == 0 else f1
                fout = f1 if h == 0 else fX
                table = gfull[h]
                cur_idx = {}
                for c4 in range(NCHUNK):
                    it = ipool.tile([128, chunk_cols[c4]], i16, tag=f"idxc{c4}")
                    nc.sync.dma_start(out=it[:], in_=idx_ext[c4][:])
                    cur_idx[c4] = it
                by_chunk = [[] for _ in range(NCHUNK)]
                for ci, cl in enumerate(calls):
                    by_chunk[cl[0]].append((ci, cl))
                ilv = []
                i = 0
                while any(by_chunk):
                    for c4 in range(NCHUNK):
                        if i < len(by_chunk[c4]):
                            ilv.append(by_chunk[c4][i])
                    i += 1
                    if all(i >= len(bc) for bc in by_chunk):
                        break
                for ci, (c4, kb, b0, nb, nidx) in ilv:
                    S = nidx // NP
                    dst_t = gpool.tile([NP, MAX_IDX_CALL // NP, H], f32, tag="gdst")
                    nc.gpsimd.dma_gather(
                        dst_t[:, :S, :],
                        table[c4 * CHROWS : (c4 + 1) * CHROWS, :],
                        cur_idx[c4][:, call_col[ci] : call_col[ci] + nidx // 16],
                        nidx,
                        nidx,
                        H,
                        single_packet=False,
                        queue_num=c4,
                    )
                    red = wpool.tile([NP, MAX_NB * H], f32, tag="red")
                    nc.vector.tensor_reduce(
                        out=red[:, : nb * H].rearrange("p (b d) -> p b d", b=nb),
                        in_=dst_t[:, :S, :].rearrange("p (b k) d -> p b d k", b=nb, k=kb),
                        axis=AX.X,
                        op=OP.add,
                    )
                    nc.scalar.dma_start(
                        out=agg_v[c4][:, b0 : b0 + nb, :],
                        in_=red[:, : nb * H].rearrange("p (b d) -> p b d", b=nb),
                    )
                for c4 in range(NCHUNK):
                    for b in range(NBLK):
                        if K[c4][b] == 0:
                            nc.sync.dma_start(
                                out=aggdram[c4][b * NP : (b + 1) * NP, :], in_=zblk[:]
                            )
                HB = NBLK // 2  # 49
                for half in range(2):
                    hb0 = half * HB
                    nhb = HB if half == 0 else NBLK - HB
                    for c4 in range(NCHUNK):
                        rt = rpool.tile([NP, NBLK - HB, H], f32, tag="rt")
                        coff = c4 * (NPAD // 16) + hb0 * (NP // 16)
                        nc.gpsimd.dma_gather(
                            rt[:, :nhb, :],
                            aggdram[c4][:, :],
                            ridx_t[:, coff : coff + nhb * (NP // 16)],
                            nhb * NP,
                            nhb * NP,
                            H,
                            single_packet=False,
                            queue_num=c4,
                        )
                        rtf = rt[:, :nhb, :].rearrange("p b d -> p (b d)")
                        fxs = fX[:, hb0 * H : (hb0 + nhb) * H]
                        if c4 == 0:
                            nc.vector.tensor_copy(fxs, rtf)
                        else:
                            nc.vector.tensor_tensor(out=fxs, in0=fxs, in1=rtf, op=OP.add)
                fX3 = fX[:].rearrange("p (b d) -> p b d", b=NBLK)
                nc.vector.tensor_tensor(out=fX3, in0=fX3, in1=dbc, op=OP.mult)
                nc.vector.tensor_tensor(out=fout[:], in0=fprev[:], in1=fX[:], op=OP.subtract)
                if h == 0:
                    for b in range(NBLK):
                        gb = wpool.tile([NP, H], f32, tag="gb")
                        nc.vector.tensor_scalar(
                            out=gb[:], in0=f1[:, b * H : (b + 1) * H],
                            scalar1=dinvT[:, b : b + 1], scalar2=None, op0=OP.mult,
                        )
                        nc.scalar.dma_start(out=gloc_v[1][:, b, :], in_=gb[:])
                    nc.sync.dma_start(out=gloc[1][NPAD : NPAD + 1, :], in_=zrow[:])
                    nc.gpsimd.collective_compute(
                        "AllGather", OP.bypass, replica_groups=rg,
                        ins=[gloc[1].ap().opt()], outs=[gfull[1].ap().opt()],
                    )

            # ---------- phase E: L3 + L4 ----------
            f2 = fX
            for bp in range(NBLK):
                ps3 = pspool.tile([NP, H], f32, tag="pB", space="PSUM")
                for k, (fk, mk) in enumerate(zip((f0, f1, f2), ("M0t", "M1t", "M2t"))):
                    pst = pspool.tile([NP, NP], f32, tag="pA", space="PSUM")
                    nc.tensor.transpose(
                        out=pst[:H, :], in_=fk[:, bp * H : (bp + 1) * H], identity=ident[:]
                    )
                    ft = wpool.tile([H, NP], f32, tag="ft")
                    nc.vector.tensor_copy(ft[:], pst[:H, :])
                    nc.tensor.matmul(
                        out=ps3[:], lhsT=ft[:], rhs=W[mk][:], start=(k == 0), stop=False,
                    )
                nc.tensor.matmul(
                    out=ps3[:], lhsT=ones_r[:], rhs=W["b3r"][:], start=False, stop=True
                )
                h3 = wpool.tile([NP, H], f32, tag="h3")
                nc.vector.tensor_scalar(out=h3[:], in0=ps3[:], scalar1=0.0, scalar2=None, op0=OP.max)
                psh = pspool.tile([NP, NP], f32, tag="pA", space="PSUM")
                nc.tensor.transpose(out=psh[:H, :], in_=h3[:], identity=ident[:])
                h3t = wpool.tile([H, NP], f32, tag="h3t")
                nc.vector.tensor_copy(h3t[:], psh[:H, :])
                ps4 = pspool.tile([NP, C], f32, tag="pC", space="PSUM")
                nc.tensor.matmul(out=ps4[:], lhsT=h3t[:], rhs=W["W4t"][:], start=True, stop=False)
                nc.tensor.matmul(out=ps4[:], lhsT=ones_r[:], rhs=W["b4r"][:], start=False, stop=True)
                nc.vector.tensor_copy(outb[:, bp * C : (bp + 1) * C], ps4[:])

            nc.sync.dma_start(out=out_ext[:], in_=outb[:])

    nc.compile()
    return nc


def kernel(**inputs):
    import concourse.bass_utils as bass_utils

    in_feat = np.asarray(inputs["in_feat"], dtype=np.float32)
    src = np.asarray(inputs["src"]).astype(np.int64)
    dst = np.asarray(inputs["dst"]).astype(np.int64)

    (calls, call_col, chunk_cols, K, idx_inputs, ridx_inputs, xt_in, dinv_in) = (
        _host_prep(in_feat, src, dst)
    )
    weights = _weights(
        np.asarray(inputs["W1"]), np.asarray(inputs["b1"]),
        np.asarray(inputs["W2"]), np.asarray(inputs["b2"]),
        np.asarray(inputs["W3"]), np.asarray(inputs["b3"]),
        np.asarray(inputs["W4"]), np.asarray(inputs["b4"]),
    )

    nc = _build_program(calls, call_col, chunk_cols, K)

    in_maps = []
    for c in range(M):
        im = {"xt": xt_in[c], "dinvT": dinv_in[c], "ridx": ridx_inputs[c]}
        for c4 in range(NCHUNK):
            im[f"idx{c4}"] = idx_inputs[c][c4]
        im.update(weights)
        in_maps.append(im)

    trace = bool(int(os.environ.get("BWGNN_TRACE", "0")))
    res = bass_utils.run_bass_kernel_spmd(nc, in_maps, list(range(M)), trace=trace)
    global LAST_EXEC_NS
    LAST_EXEC_NS = res.exec_time_ns

    full = np.empty((N, C), dtype=np.float32)
    for c in range(M):
        r = (
            res.results[c]["out"]
            .reshape(NP, NBLK, C)
            .transpose(1, 0, 2)
            .reshape(NPAD, C)
        )
        full[c * NL : (c + 1) * NL] = r[:NL]
    return full
